# revision 30
# baseline (speedup 1.0000x reference)
"""BiMamba block Trainium2 kernel.

Sharding: 8 cores = (2 directions) x (4 batches). Stage 1 runs a full Mamba
direction for one batch per core with zero inter-core communication; stage 2
(second launch) combines forward/backward via the sigmoid gate and applies
the final projection; 8 cores = (4 batches) x (2 token halves).

Stage 1 is a software-pipelined 4-chunk (Q=256) loop over time. Per fused
d-tile iteration, trace order interleaves (so each engine's in-order queue
overlaps work):
  DVE:    dbu, carry-inject, B-mult, ONE merged 16-state scan, C-mult, gate
  PE:     z-proj GEMM, next-iter dt_proj, next-chunk in_proj + causal conv
          (diagonal-stationary matmuls, PSUM-accumulated), x_proj, 16->1
          state reduction (identity matmuls) + Dp skip, out_proj accumulation
  Scalar: silu/exp/softplus chain, 16 dA exps (one per state), PSUM copies
The Vector engine is the bottleneck (scan = 2 cyc/elem serial; B/C mults at
the tensor_tensor 2x cap), so everything else hides under it. Scan state
crosses chunk boundaries through an injected column: dA col 0 is
exp(A*1e30)=0 (poisoned delta cuts the merged-state segments) and dbub col 0
carries h from the previous chunk, so state := carry exactly. delta/dA for
iteration d+1 are produced during iteration d; in_proj/conv/x_proj for chunk
c+1 run during chunk c's scans (stage-A d-tiles lead by one at d=14 so the
B/C DRAM-bounce broadcast latency is covered). PSUM: 4 banks out_proj
accumulators (8 m-tiles in column halves) + 1 x_proj + 1 stage-A (u|conv
halves) + 1 scan-phase (z|dt halves) + 1 state-reduce = 8.
Layout: channel-on-partition, time on the free dim. GpSimd runs nothing
elementwise (its SBUF port arbitrates against the DVE's second read port).
"""

import numpy as np
import ml_dtypes

import concourse.bass as bass
from concourse import bacc
import concourse.tile as tile
import concourse.mybir as mybir
from concourse.bass_utils import run_bass_kernel_spmd

F32 = mybir.dt.float32
BF16 = mybir.dt.bfloat16
F8 = mybir.dt.float8e4
DR = mybir.MatmulPerfMode.DoubleRow
AF = mybir.ActivationFunctionType
OP = mybir.AluOpType
ts = bass.ts

D_MODEL = 1024
D_INNER = 2048
D_STATE = 16
D_CONV = 4
DT_RANK = 64
BATCH = 4
SEQ = 1024

NDT = D_INNER // 128  # 16 d-tiles
NKT = D_MODEL // 128  # 8 k-tiles over d_model
NB = np.dtype(ml_dtypes.bfloat16)
F8NP = np.dtype(ml_dtypes.float8_e4m3)

Q = 256                # tokens per chunk
NCH = SEQ // Q         # 4 chunks
QP = Q + 1             # +1 carry-injection column per state segment


def _pair8(w, npair, scale):
    # [K, M] -> [128, npair*2*M] fp8, rows pair-interleaved for DoubleRow
    K, M = w.shape
    q = np.clip(np.asarray(w, np.float32) * scale, -240.0, 240.0)
    return np.ascontiguousarray(
        q.reshape(npair, 2, 128, M).transpose(2, 0, 1, 3)
    ).reshape(128, npair * 2 * M).astype(F8NP)


def build_stage1():
    nc = bacc.Bacc("TRN2", target_bir_lowering=False, debug=False, num_devices=8)

    xT = nc.dram_tensor("xT", [D_MODEL, SEQ], BF16, kind="ExternalInput")
    wu = nc.dram_tensor("wu", [D_MODEL, D_INNER], BF16, kind="ExternalInput")
    wz = nc.dram_tensor("wz", [D_MODEL, D_INNER], BF16, kind="ExternalInput")
    conv_diag = nc.dram_tensor(
        "conv_diag", [128, NDT * D_CONV * 128], BF16, kind="ExternalInput"
    )
    conv_b = nc.dram_tensor("conv_b", [128, NDT], F32, kind="ExternalInput")
    xproj = nc.dram_tensor("xproj", [D_INNER, 96], BF16, kind="ExternalInput")
    dt_w = nc.dram_tensor("dt_w", [DT_RANK, D_INNER], BF16, kind="ExternalInput")
    dt_b = nc.dram_tensor("dt_b", [128, NDT], F32, kind="ExternalInput")
    A_in = nc.dram_tensor("A", [128, NDT * D_STATE], F32, kind="ExternalInput")
    outproj = nc.dram_tensor("outproj", [D_INNER, D_MODEL], BF16, kind="ExternalInput")
    ident_in = nc.dram_tensor("ident", [128, 128], BF16, kind="ExternalInput")
    dpd_in = nc.dram_tensor("dp_diag", [128, NDT * 128], BF16, kind="ExternalInput")

    y_dir = nc.dram_tensor("y_dir", [D_MODEL, SEQ], BF16, kind="ExternalOutput")

    from contextlib import ExitStack
    with tile.TileContext(nc) as tc:
        with ExitStack() as _es:
            pool = lambda n, b, **kw: _es.enter_context(
                tc.tile_pool(name=n, bufs=b, **kw)
            )
            consts = pool("consts", 1)
            persist = pool("persist", 1)
            dram = pool("dram", 1, space="DRAM")
            wst = pool("wst", 2)
            ops = pool("ops", 2)
            ucp = pool("ucp", 1)
            bcp = pool("bcp", 2)
            dblp = pool("dblp", 2)
            esp = pool("esp", 8)
            dbup = pool("dbup", 1)
            hcp = pool("hcp", 1)
            dap = pool("dap", 2)
            dbp = pool("dbp", 1)
            ytp = pool("ytp", 2)
            obp = pool("obp", 1)
            # PSUM (8 banks, one accumulation group per bank):
            # psSA: u, cv | psSC: z, dt | psY: y0, y1 | psB: dbl, op
            psSA = pool("psSA", 1, space="PSUM")
            psSC = pool("psSC", 1, space="PSUM")
            psB = pool("psB", 1, space="PSUM")
            psY = pool("psY", 1, space="PSUM")
            cb = consts.tile([128, NDT], F32)
            nc.sync.dma_start(cb[:], conv_b[:])
            dtb = consts.tile([128, NDT], F32)
            nc.sync.dma_start(dtb[:], dt_b[:])
            A_sb = consts.tile([128, NDT * D_STATE], F32)
            nc.sync.dma_start(A_sb[:], A_in[:])
            dtw_sb = consts.tile([DT_RANK, D_INNER], BF16)
            nc.sync.dma_start(dtw_sb[:], dt_w[:])
            ident = consts.tile([128, 128], BF16)
            nc.sync.dma_start(ident[:], ident_in[:])
            dpd = consts.tile([128, NDT * 128], BF16)
            nc.sync.dma_start(dpd[:], dpd_in[:])
            cvd = consts.tile([128, NDT, D_CONV, 128], BF16)
            nc.sync.dma_start(
                cvd[:], conv_diag.ap().rearrange(
                    "p (d k m) -> p d k m", d=NDT, k=D_CONV
                )
            )
            xp_sb = consts.tile([128, NDT, 96], BF16)
            nc.sync.dma_start(
                xp_sb[:], xproj.ap().rearrange("(dt p) f -> p dt f", p=128)
            )

            xt_sb = persist.tile([128, NKT, SEQ], BF16)
            for c in range(NCH):
                nc.sync.dma_start(
                    xt_sb[:, :, c * Q : (c + 1) * Q],
                    xT.ap()[:, c * Q : (c + 1) * Q].rearrange(
                        "(kt p) t -> p kt t", p=128
                    ),
                )
            # per-d-tile u with a 4-column history head for the causal conv
            u_sb = [persist.tile([128, 4 + Q], BF16, name=f"u{d}") for d in range(NDT)]
            for d in range(NDT):
                nc.vector.memset(u_sb[d][:, 0:4], 0.0)
            # last-column scan state per (d-tile, state): carries across chunks
            hlast = persist.tile([128, NDT, D_STATE], BF16)
            nc.vector.memset(hlast[:].rearrange("p d n -> p (d n)"), 0.0)

            uc_all = [
                ucp.tile([128, NDT, Q], BF16, name=f"uca{i}") for i in range(2)
            ]
            yg_all = [
                ucp.tile([128, NDT, Q], BF16, name=f"yga{i}") for i in range(2)
            ]
            szx_all = [
                ucp.tile([128, NDT, Q], BF16, name=f"sza{i}") for i in range(2)
            ]
            dbl_bc = [
                dram.tile([2 * D_STATE, Q], BF16, name=f"dblbc{c}")
                for c in range(NCH)
            ]
            # PSUM rule: a start=True matmul resets its WHOLE bank, so every
            # accumulation group owns a full bank. 8 banks total:
            # u, conv, z, dt, y0, y1 (d-parity), x_proj, out_proj.
            psy_pair = [
                psY.tile([128, Q], F32, tag=f"y{i}", name=f"psy{i}")
                for i in range(2)
            ]
            # pre-poisoned delta buffers (col 0 = 1e30 forever): Ln only ever
            # writes cols 1..Q, so no per-iteration DVE memset is needed
            delta_pair = [
                persist.tile([128, QP], BF16, name=f"dpp{i}") for i in range(10)
            ]
            for i in range(10):
                nc.vector.memset(delta_pair[i][:, 0:1], 1.0e30)

            chunk_bc = {}   # c -> (dbl, B_rep, C_rep)
            delta_t = {}    # (c, d) -> (delta, dA)
            n_prod = [0]

            def stage_a_d(c, d):
                sl = slice(c * Q, (c + 1) * Q)
                ucx = uc_all[c % 2]
                w2 = wst.tile([128, NKT, 128], BF16, tag="w2", name=f"w2_{c}_{d}")
                nc.sync.dma_start(
                    w2[:], wu.ap()[:, ts(d, 128)].rearrange(
                        "(kt p) m -> p kt m", p=128
                    ),
                )
                if c > 0:
                    # conv history: last 3 tokens of the previous chunk
                    nc.scalar.activation(
                        u_sb[d][:, 0:4], u_sb[d][:, Q : Q + 4], AF.Copy
                    )
                ups = psSA.tile([128, Q], F32, tag="u", name=f"u_{c}_{d}")
                for k in range(NKT):
                    nc.tensor.matmul(
                        ups[:], w2[:, k], xt_sb[:, k, sl],
                        start=(k == 0), stop=(k == NKT - 1),
                    )
                nc.scalar.activation(u_sb[d][:, 4 : 4 + Q], ups[:], AF.Copy)
                # causal depthwise conv on the PE: accumulate
                # diag(w_k) @ u[shifted] taps into PSUM
                cps = psSA.tile([128, Q], F32, tag="cv", name=f"cv_{c}_{d}")
                for k in range(D_CONV):
                    nc.tensor.matmul(
                        cps[:], cvd[:, d, k],
                        u_sb[d][:, k + 1 : k + 1 + Q],
                        start=(k == 0), stop=(k == D_CONV - 1),
                    )
                nc.scalar.activation(
                    ucx[:, d, :], cps[:], AF.Silu, bias=cb[:, d : d + 1]
                )
                dbl_ps = chunk_ps[c]
                nc.tensor.matmul(
                    dbl_ps[0:96, :], xp_sb[:, d], ucx[:, d, :],
                    start=(d == 0), stop=(d == NDT - 1),
                )
                # z-projection + silu here too, so ALL scalar silus batch in
                # stage A and the scan-phase scalar mix stays within the
                # exp/ln/copy table set (act-table load thrash removal)
                wzt = wst.tile([128, NKT, 128], BF16, tag="wz", name=f"wz_{c}_{d}")
                nc.sync.dma_start(
                    wzt[:],
                    wz.ap()[:, ts(d, 128)].rearrange("(kt p) m -> p kt m", p=128),
                )
                zps = psSC.tile([128, Q], F32, tag="z", name=f"z_{c}_{d}")
                for k in range(NKT):
                    nc.tensor.matmul(
                        zps[:], wzt[:, k], xt_sb[:, k, sl],
                        start=(k == 0), stop=(k == NKT - 1),
                    )
                nc.scalar.activation(szx_all[c % 2][:, d, :], zps[:], AF.Silu)

            def dbl_finish(c):
                dbl = dblp.tile([96, Q], BF16, tag="dbl", name=f"dbl_{c}")
                nc.scalar.activation(dbl[:], chunk_ps[c][0:96, :], AF.Copy)
                # replicate B/C rows across partitions via DRAM-bounce DMAs
                nc.sync.dma_start(dbl_bc[c][:], dbl[64 : 64 + 2 * D_STATE, :])
                B_rep = bcp.tile([128, D_STATE, Q], BF16, tag="B", name=f"B_{c}")
                C_rep = bcp.tile([128, D_STATE, Q], BF16, tag="C", name=f"C_{c}")
                for half in range(2):
                    nc.sync.dma_start(
                        B_rep[:, ts(half, 8), :],
                        dbl_bc[c][half * 8 : (half + 1) * 8, :].rearrange(
                            "(o n) t -> o n t", o=1
                        ).broadcast_to([128, 8, Q]),
                    )
                    nc.scalar.dma_start(
                        C_rep[:, ts(half, 8), :],
                        dbl_bc[c][
                            D_STATE + half * 8 : D_STATE + (half + 1) * 8, :
                        ].rearrange("(o n) t -> o n t", o=1).broadcast_to(
                            [128, 8, Q]
                        ),
                    )
                chunk_bc[c] = (dbl, B_rep, C_rep)

            def produce_dl(items):
                # dt_proj + softplus(x) = Ln(Exp(x + dt_b) + 1); delta col 0
                # is the persistent 1e30 poison. Batched 8 d-tiles ahead with
                # the Lns grouped so the exp<->ln table-set switch happens
                # twice per 8 iterations instead of twice per iteration.
                esbs = []
                for c, d in items:
                    dbl = chunk_bc[c][0]
                    dps = psSC.tile([128, Q], F32, tag="dt", name=f"dt_{c}_{d}")
                    nc.tensor.matmul(
                        dps[:], dtw_sb[:, ts(d, 128)], dbl[0:DT_RANK, :],
                        start=True, stop=True,
                    )
                    esb = esp.tile([128, Q], BF16, tag="esb", name=f"es_{c}_{d}")
                    nc.scalar.activation(
                        esb[:], dps[:], AF.Exp, bias=dtb[:, d : d + 1]
                    )
                    esbs.append(esb)
                for (c, d), esb in zip(items, esbs):
                    delta = delta_pair[n_prod[0] % 10]
                    n_prod[0] += 1
                    nc.scalar.activation(delta[:, 1:QP], esb[:], AF.Ln, bias=1.0)
                    delta_t[(c, d)] = delta

            def produce_dA(c, d):
                # the 16 per-state dA = exp(A_n * delta), two iterations
                # ahead: uniform per-iteration scalar load, all in the exp
                # table set, with two iterations of slack before the scan
                delta = delta_t[(c, d)]
                dA = dap.tile(
                    [128, D_STATE, QP], BF16, tag="dA", name=f"dA_{c}_{d}",
                    bufs=3,
                )
                for n in range(D_STATE):
                    nc.scalar.activation(
                        dA[:, n, :], delta[:], AF.Exp,
                        scale=A_sb[:, d * D_STATE + n : d * D_STATE + n + 1],
                    )
                delta_t[(c, d)] = (delta, dA)

            def scan_front(c, d):
                ucx = uc_all[c % 2]
                delta, dA = delta_t.pop((c, d))
                B_rep = chunk_bc[c][1]
                dbub = dbp.tile([128, D_STATE, QP], BF16, tag="dbub")
                dbu = dbup.tile([128, Q], BF16, tag="dbu", name=f"db_{c}_{d}")
                nc.vector.tensor_tensor(
                    dbu[:], delta[:, 1:QP], ucx[:, d, :], OP.mult
                )
                # carry-injection column, then B-mult into cols 1..Q
                nc.vector.tensor_copy(
                    dbub[:, :, 0:1].rearrange("p n o -> p (n o)"),
                    hlast[:, d, :],
                )
                nc.vector.tensor_tensor(
                    dbub[:, :, 1:QP],
                    dbu[:].rearrange("p (g t) -> p g t", g=1).broadcast_to(
                        [128, D_STATE, Q]
                    ),
                    B_rep[:], OP.mult,
                )
                # the scan writes h IN PLACE over dA: the recurrence is
                # strictly serial, so element t's write never precedes its
                # own reads, and dA is dead after the scan
                nc.vector.tensor_tensor_scan(
                    dA[:].rearrange("p n t -> p (n t)"),
                    dA[:].rearrange("p n t -> p (n t)"),
                    dbub[:].rearrange("p n t -> p (n t)"),
                    0.0, OP.mult, OP.add,
                )
                return dA, dbub, szx_all[c % 2][:, d, :]

            def scan_back(c, d, h):
                ucx = uc_all[c % 2]
                C_rep = chunk_bc[c][2]
                hv = h[:]
                nc.vector.tensor_copy(
                    hlast[:, d, :],
                    hv[:, :, Q : Q + 1].rearrange("p n o -> p (n o)"),
                )
                hct = hcp.tile([128, D_STATE, Q], BF16, tag="hc", name="hc")
                hc = hct[:]
                nc.vector.tensor_tensor(hc, hv[:, :, 1:QP], C_rep[:], OP.mult)
                # 16 -> 1 state reduction + Dp*uc skip term on the PE
                psy = psy_pair[d % 2][:]
                for n in range(D_STATE):
                    nc.tensor.matmul(
                        psy, ident[:], hc[:, n, :],
                        start=(n == 0), stop=False,
                    )
                nc.tensor.matmul(
                    psy, dpd[:, ts(d, 128)], ucx[:, d, :],
                    start=False, stop=True,
                )
                return psy

            def ytot_prev(c, d, psy, szx):
                # deferred by one iteration: the reduce -> ytot -> yg chain
                # never stalls the DVE (ytot runs first in the next
                # iteration's scalar queue, yg after that iteration's scan)
                ytot = ytp.tile([128, Q], BF16, tag="ytot")
                nc.scalar.activation(ytot[:], psy, AF.Copy)
                return ytot

            def yg_prev(c, d, ytot, szx):
                yg = yg_all[c % 2][:, d, :]
                nc.vector.tensor_tensor(yg, ytot[:], szx, OP.mult)

            def out_proj_m(c, m):
                # chunk-deferred out_proj: one m-tile column band, contracted
                # over all d into a single psum bank (sequential groups)
                opwm = ops.tile([128, NDT, 128], BF16, tag="opw", name=f"opw_{c}_{m}")
                nc.gpsimd.dma_start(
                    opwm[:],
                    outproj.ap()[:, ts(m, 128)].rearrange(
                        "(dt p) m -> p dt m", p=128
                    ),
                )
                ygc = yg_all[c % 2]
                po = psB.tile([128, Q], F32, tag="op", name=f"po_{c}_{m}")
                for d in range(NDT):
                    nc.tensor.matmul(
                        po[:], opwm[:, d, :], ygc[:, d, :],
                        start=(d == 0), stop=(d == NDT - 1),
                    )
                ob = obp.tile([128, Q], BF16, tag="ob", name=f"ob_{c}_{m}")
                nc.scalar.activation(ob[:], po[:], AF.Copy)
                nc.scalar.dma_start(
                    y_dir.ap()[ts(m, 128), c * Q : (c + 1) * Q], ob[:]
                )

            chunk_ps = [
                psB.tile([128, Q], F32, tag="dbl", name=f"dblps{c}")
                for c in range(NCH)
            ]

            def glob_cd(i):
                # global iteration index -> (chunk, d-tile), or None past end
                if i < 0 or i >= NCH * NDT:
                    return None
                return (i // NDT, i % NDT)

            # prologue: stage A for chunk 0, then deltas 8 ahead, dA 2 ahead
            for d in range(NDT):
                stage_a_d(0, d)
            dbl_finish(0)
            produce_dl([(0, d) for d in range(8)])
            produce_dA(0, 0)
            produce_dA(0, 1)
            assert NDT % 8 == 0

            pend = None  # deferred (c, d, psy, szx)
            for c in range(NCH):
                for d in range(NDT):
                    gi = c * NDT + d
                    if pend is not None:
                        yt = ytot_prev(*pend)
                    h, dbub, szx = scan_front(c, d)
                    # dA exps two iterations ahead (uniform scalar load)
                    nxt = glob_cd(gi + 2)
                    if nxt is not None:
                        produce_dA(*nxt)
                    # next chunk's stage A in blocks of 4 d-tiles, so its
                    # silus batch into at most 2 act-table switches per block;
                    # finished by iteration 12 to cover broadcast latency
                    if c + 1 < NCH and d % 4 == 0 and d < NDT - 3:
                        for dd in range(d, d + 4):
                            stage_a_d(c + 1, dd)
                        if d == 12:
                            dbl_finish(c + 1)
                    # softplus deltas in batches of 8, 4..11 iterations
                    # ahead (after the stage-A block: chunk c+1's dbl is
                    # finished at iteration 12, just before its first use)
                    if gi % 8 == 4:
                        items = [glob_cd(i) for i in range(gi + 4, gi + 12)]
                        items = [x for x in items if x is not None]
                        if items:
                            produce_dl(items)
                    # previous chunk's out_proj, one m-tile per odd iteration
                    if c > 0 and d % 2 == 1:
                        out_proj_m(c - 1, d // 2)
                    psy = scan_back(c, d, h)
                    if pend is not None:
                        yg_prev(pend[0], pend[1], yt, pend[3])
                    pend = (c, d, psy, szx)
            yt = ytot_prev(*pend)
            yg_prev(pend[0], pend[1], yt, pend[3])
            for m in range(8):
                out_proj_m(NCH - 1, m)

    nc.compile()
    return nc


def build_stage2():
    nc = bacc.Bacc("TRN2", target_bir_lowering=False, debug=False, num_devices=8)

    TH = SEQ // 2
    yA = nc.dram_tensor("yA", [D_MODEL, TH], BF16, kind="ExternalInput")
    yB = nc.dram_tensor("yB", [D_MODEL, TH], BF16, kind="ExternalInput")
    yA8 = nc.dram_tensor("yA8", [128, 4 * 2 * TH], F8, kind="ExternalInput")
    yB8 = nc.dram_tensor("yB8", [128, 4 * 2 * TH], F8, kind="ExternalInput")
    gwA8 = nc.dram_tensor("gwA8", [128, 4 * 2 * D_MODEL], F8, kind="ExternalInput")
    gwB8 = nc.dram_tensor("gwB8", [128, 4 * 2 * D_MODEL], F8, kind="ExternalInput")
    gb = nc.dram_tensor("gb", [128, NKT], F32, kind="ExternalInput")
    pw = nc.dram_tensor("pw", [D_MODEL, D_MODEL], BF16, kind="ExternalInput")
    pb = nc.dram_tensor("pb", [128, NKT], F32, kind="ExternalInput")

    out = nc.dram_tensor("out", [D_MODEL, TH], F32, kind="ExternalOutput")

    with tile.TileContext(nc) as tc:
        with (
            tc.tile_pool(name="sb", bufs=1) as sb,
            tc.tile_pool(name="wst", bufs=4) as wst,
            tc.tile_pool(name="tmp", bufs=3) as tmp,
            tc.tile_pool(name="ps", bufs=4, space="PSUM") as ps,
        ):
            gb_sb = sb.tile([128, NKT], F32)
            nc.sync.dma_start(gb_sb[:], gb[:])
            pb_sb = sb.tile([128, NKT], F32)
            nc.sync.dma_start(pb_sb[:], pb[:])
            ya_sb = sb.tile([128, NKT, TH], BF16)
            nc.sync.dma_start(
                ya_sb[:], yA.ap().rearrange("(kt p) t -> p kt t", p=128)
            )
            yb_sb = sb.tile([128, NKT, TH], BF16)
            nc.sync.dma_start(
                yb_sb[:], yB.ap().rearrange("(kt p) t -> p kt t", p=128)
            )
            ya8_sb = sb.tile([128, 4, 2, TH], F8)
            nc.sync.dma_start(
                ya8_sb[:], yA8.ap().rearrange("p (k j t) -> p k j t", k=4, j=2)
            )
            yb8_sb = sb.tile([128, 4, 2, TH], F8)
            nc.sync.dma_start(
                yb8_sb[:], yB8.ap().rearrange("p (k j t) -> p k j t", k=4, j=2)
            )
            yc_sb = sb.tile([128, NKT, TH], BF16)
            gwa_all = sb.tile([128, 4, 2, D_MODEL], F8)
            nc.sync.dma_start(
                gwa_all[:],
                gwA8.ap().rearrange("p (k j m) -> p k j m", k=4, j=2),
            )
            gwb_all = sb.tile([128, 4, 2, D_MODEL], F8)
            nc.sync.dma_start(
                gwb_all[:],
                gwB8.ap().rearrange("p (k j m) -> p k j m", k=4, j=2),
            )
            pw_all = sb.tile([128, NKT, NKT, 128], BF16)
            nc.sync.dma_start(
                pw_all[:],
                pw.ap().rearrange("(kt p) (mt m) -> p kt mt m", p=128, m=128),
            )
            for m in range(NKT):
                gps = ps.tile([128, TH], F32, tag="g")
                for k in range(4):
                    nc.tensor.matmul(
                        gps[:], gwa_all[:, k, :, ts(m, 128)], ya8_sb[:, k],
                        start=(k == 0), stop=False, perf_mode=DR,
                    )
                for k in range(4):
                    nc.tensor.matmul(
                        gps[:], gwb_all[:, k, :, ts(m, 128)], yb8_sb[:, k],
                        start=False, stop=(k == 3), perf_mode=DR,
                    )
                g = tmp.tile([128, TH], BF16, tag="gg")
                nc.scalar.activation(
                    g[:], gps[:], AF.Sigmoid, scale=1.0 / 8192.0,
                    bias=gb_sb[:, m : m + 1]
                )
                # y = yB + g*(yA - yB)
                dsub = tmp.tile([128, TH], BF16, tag="dsub")
                nc.vector.tensor_tensor(
                    dsub[:], ya_sb[:, m, :], yb_sb[:, m, :], OP.subtract
                )
                gm = tmp.tile([128, TH], BF16, tag="gm")
                nc.vector.tensor_tensor(gm[:], g[:], dsub[:], OP.mult)
                nc.vector.tensor_tensor(
                    yc_sb[:, m, :], yb_sb[:, m, :], gm[:], OP.add
                )
            for m2 in range(NKT):
                pps = ps.tile([128, TH], F32, tag="p")
                for k in range(NKT):
                    nc.tensor.matmul(
                        pps[:], pw_all[:, k, m2, :], yc_sb[:, k, :],
                        start=(k == 0), stop=(k == NKT - 1),
                    )
                ob = tmp.tile([128, TH], F32, tag="ob")
                nc.scalar.activation(
                    ob[:], pps[:], AF.Identity, bias=pb_sb[:, m2 : m2 + 1]
                )
                nc.sync.dma_start(out.ap()[ts(m2, 128)], ob[:])

    nc.compile()
    return nc


def _tile_vec(v, nt):
    return np.ascontiguousarray(np.asarray(v, np.float32).reshape(nt, 128).T)


_CACHE = {}


def kernel(**inputs):
    inputs = {k: np.asarray(v) for k, v in inputs.items()}
    if "s1" not in _CACHE:
        _CACHE["s1"] = build_stage1()
        _CACHE["s2"] = build_stage2()
    nc1, nc2 = _CACHE["s1"], _CACHE["s2"]

    x = inputs["x"].astype(np.float32)  # [B, L, D]

    ident_np = np.eye(128, dtype=np.float32).astype(NB)

    def _diag_tiles(w):
        # w: [NDT, 128] per-channel scalars -> [128, NDT*128] diag blocks
        out = np.zeros((128, w.shape[0], 128), np.float32)
        for di in range(w.shape[0]):
            np.fill_diagonal(out[:, di, :], w[di])
        return out.reshape(128, w.shape[0] * 128).astype(NB)

    def _conv_diag(cw):
        # cw: [D_INNER, D_CONV] -> [128, NDT*D_CONV*128] diag(w_k) blocks
        cw = np.asarray(cw, np.float32).reshape(NDT, 128, D_CONV)
        out = np.zeros((128, NDT, D_CONV, 128), np.float32)
        for di in range(NDT):
            for k in range(D_CONV):
                np.fill_diagonal(out[:, di, k, :], cw[di, :, k])
        return out.reshape(128, NDT * D_CONV * 128).astype(NB)

    maps1 = []
    for core in range(8):
        s = "f" if core < 4 else "b"
        b = core % 4
        xb = x[b]
        if s == "b":
            xb = xb[::-1]
        inproj = inputs[f"inproj_{s}"].astype(np.float32)
        maps1.append(
            dict(
                xT=np.ascontiguousarray(xb.T).astype(NB),
                wu=inproj[:, :D_INNER].astype(NB),
                wz=inproj[:, D_INNER:].astype(NB),
                conv_diag=_conv_diag(inputs[f"conv_w_{s}"]),
                conv_b=_tile_vec(inputs[f"conv_b_{s}"], NDT),
                xproj=inputs[f"xproj_{s}"].astype(NB),
                dt_w=inputs[f"dt_w_{s}"].astype(NB),
                dt_b=_tile_vec(inputs[f"dt_b_{s}"], NDT),
                A=np.ascontiguousarray(
                    (-np.exp(np.asarray(inputs[f"Alog_{s}"], np.float32)))
                    .reshape(NDT, 128, D_STATE)
                    .transpose(1, 0, 2)
                    .reshape(128, NDT * D_STATE)
                ),
                outproj=inputs[f"outproj_{s}"].astype(NB),
                ident=ident_np,
                dp_diag=_diag_tiles(
                    np.asarray(inputs[f"Dp_{s}"], np.float32).reshape(NDT, 128)
                ),
            )
        )
    global _last_maps1
    _last_maps1 = maps1
    res1 = run_bass_kernel_spmd(nc1, maps1, list(range(8)))
    y_dirs = [res1.results[c]["y_dir"] for c in range(8)]  # [D_MODEL, SEQ] bf16
    for c in range(4, 8):
        y_dirs[c] = y_dirs[c][:, ::-1]

    gate_w = inputs["gate_w"].astype(np.float32)
    gwA8 = _pair8(gate_w[:D_MODEL], 4, 64.0)
    gwB8 = _pair8(gate_w[D_MODEL:], 4, 64.0)
    gb = _tile_vec(inputs["gate_b"], NKT)
    pw = inputs["proj_w"].astype(NB)
    pb = _tile_vec(inputs["proj_b"], NKT)

    maps2 = []
    for core in range(8):
        b = core % 4
        half = core // 4
        sl = slice(half * 512, (half + 1) * 512)
        ya = np.ascontiguousarray(y_dirs[b][:, sl])
        yb = np.ascontiguousarray(y_dirs[4 + b][:, sl])
        maps2.append(
            dict(
                yA=ya,
                yB=yb,
                yA8=_pair8(ya.astype(np.float32), 4, 128.0),
                yB8=_pair8(yb.astype(np.float32), 4, 128.0),
                gwA8=gwA8, gwB8=gwB8, gb=gb, pw=pw, pb=pb,
            )
        )
    global _last_maps2
    _last_maps2 = maps2
    res2 = run_bass_kernel_spmd(nc2, maps2, list(range(8)))

    out = np.empty((BATCH, SEQ, D_MODEL), np.float32)
    for core in range(8):
        b = core % 4
        half = core // 4
        o = res2.results[core]["out"]  # [D_MODEL, 512] f32
        out[b, half * 512 : (half + 1) * 512, :] = o.T
    return out
